# revision 36
# baseline (speedup 1.0000x reference)
"""BiBatchHardTripletLoss on 8 Trainium2 NeuronCores.

Math (reference): inputs [8192,1024] split into rgb=inputs[:4096], ir=inputs[4096:].
  dist[i,j] = ||rgb_i - ir_j||
  mask[i,j] = (targets[j] == targets[4096+i])          (the "transposed" quirk)
  rgb_ap[i] = max_j masked dist, rgb_an[i] = min_j unmasked dist   (rows)
  ir_ap[j]  = max_i masked dist, ir_an[j]  = min_i unmasked dist   (cols)
  loss = mean(relu(.3-(rgb_an-rgb_ap))) + mean(relu(.3-(ir_an-ir_ap)))

Device strategy (data-parallel over the 4096 rgb rows, ir replicated):
  Core k computes the [512, 4096] block of squared distances (sans the
  per-row ||rgb_i||^2, which is constant along rows) plus a mask bump:
      P[i,j] = -2*rgb_i.ir_j + ||ir_j||^2 + 65536*eq[i,j]     (PSUM, fp32)
  via 11 accumulating float32r matmuls per [128,512] tile (f32r = e8m11 at
  full PE rate; all inputs pre-rounded / exactly representable):
    - 8 K-tiles of (-2*rgb_slab)^T @ ir^T            (K=1024 contraction)
    - 1 mask matmul: lhsT[l,i] = 65536*(t_ir[512k+i]==l), rhs[l,j]=(t_rgb[j]==l)
    - 1 K=2 matmul: lhsT = ones[2,128], rhs = (e8m11_hi(c2); residual_lo(c2))
      adding ||ir_j||^2 to e8m11-residual accuracy (~1e-4 abs).
  DVE row-max/min reduce P directly (PSUM) -> rgb-side stats (host adds the
  missing ||rgb_i||^2 afterwards - exact, it's constant per row).
  ACT writes S = P + ||rgb_i||^2 into SBUF; PE transposes S in 128x128 blocks
  into PSUM; DVE row-reduces those -> ir-side partials over the 512 local rows.
  Host: combine partials over cores, un-bump (max-65536), sqrt, relu, mean.
  max(sq)~2600 << 65536 so the bump cleanly separates positives.
"""

import os

import numpy as np

import concourse.bass as bass
from concourse import bacc
import concourse.mybir as mybir
import concourse.tile as tile
from concourse.bass_utils import run_bass_kernel_spmd

F32 = mybir.dt.float32
F32R = mybir.dt.float32r
BF16 = mybir.dt.bfloat16

N = 4096            # rows per side
D = 1024            # embedding dim
NCORES = 8
SLAB = N // NCORES  # 512 rgb rows per core
KT = D // 128       # 8 contraction tiles
MI = SLAB // 128    # 4 row chunks
NJG = 4             # column groups of 1024
BUMP = 65536.0

_CACHE = {}
LAST_RESULTS = None  # test.py reads exec_time_ns from here when tracing

USE_F32R = os.environ.get("K_F32R", "1") == "1"
MM_DT = F32R if USE_F32R else F32


def _build_nc():
    nc = bacc.Bacc()

    rgbT = nc.dram_tensor("rgbT", [KT, 128, SLAB], MM_DT, kind="ExternalInput")
    irT = nc.dram_tensor("irT", [KT, 128, N], MM_DT, kind="ExternalInput")
    ohr = nc.dram_tensor("ohr", [128, SLAB], BF16, kind="ExternalInput")
    ohc = nc.dram_tensor("ohc", [128, N], BF16, kind="ExternalInput")
    c2hl = nc.dram_tensor("c2hl", [3, N], BF16, kind="ExternalInput")
    ones2 = nc.dram_tensor("ones2", [3, 128], BF16, kind="ExternalInput")
    r2 = nc.dram_tensor("r2", [128, MI], F32, kind="ExternalInput")
    ident = nc.dram_tensor("ident", [128, 128], F32, kind="ExternalInput")
    o_rmax = nc.dram_tensor("rgb_max", [128, MI, NJG], F32, kind="ExternalOutput")
    o_rmin = nc.dram_tensor("rgb_min", [128, MI, NJG], F32, kind="ExternalOutput")
    o_imax = nc.dram_tensor("ir_max", [128, 8, MI, NJG], F32, kind="ExternalOutput")
    o_imin = nc.dram_tensor("ir_min", [128, 8, MI, NJG], F32, kind="ExternalOutput")

    with tile.TileContext(nc) as tc:
        with (
            tc.tile_pool(name="big", bufs=1) as big,
            tc.tile_pool(name="spool", bufs=3) as spool,
            tc.tile_pool(name="gpsum", bufs=2, space="PSUM") as gpool,
            tc.tile_pool(name="tpsum", bufs=2, space="PSUM") as tpool,
            tc.tile_pool(name="stats", bufs=1) as stats,
        ):
            # --- resident inputs ---
            # issue order matters: first compute tile needs rgbT k-chunks and
            # irT[kk][:, 0:1024]; alternate issue engine (HWDGE via sync,
            # SWDGE via gpsimd) to parallelize the serial DMA-issue streams
            s_rgbT = big.tile([128, KT, SLAB], MM_DT, name="s_rgbT", tag="rgbT")
            s_ohr = big.tile([128, SLAB], BF16, name="s_ohr", tag="ohr")
            s_r2 = big.tile([128, MI], F32, name="s_r2", tag="r2")
            s_ident = big.tile([128, 128], F32, name="s_ident", tag="ident")
            s_c2hl = big.tile([3, N], BF16, name="s_c2hl", tag="c2hl")
            s_ones2 = big.tile([3, 128], BF16, name="s_ones2", tag="ones2")
            s_ohc = big.tile([128, N], BF16, name="s_ohc", tag="ohc")
            s_irT = [
                big.tile([128, N], MM_DT, name=f"s_irT{kk}", tag=f"irT{kk}")
                for kk in range(KT)
            ]

            engines = [nc.sync, nc.gpsimd]

            def eng(kk):
                return engines[kk % 2]

            h0, h1 = slice(0, 512), slice(512, 1024)
            nc.sync.dma_start(out=s_ohr, in_=ohr[:, :])
            nc.gpsimd.dma_start(out=s_ohc[:, 0:1024], in_=ohc[:, 0:1024])
            nc.sync.dma_start(out=s_c2hl, in_=c2hl[:, :])
            nc.gpsimd.dma_start(out=s_ones2, in_=ones2[:, :])
            for kk in range(KT):
                eng(kk).dma_start(out=s_rgbT[:, kk, :], in_=rgbT[kk])
                eng(kk).dma_start(out=s_irT[kk][:, h0], in_=irT[kk, :, h0])
            for kk in range(KT):
                eng(kk).dma_start(out=s_irT[kk][:, h1], in_=irT[kk, :, h1])
            nc.sync.dma_start(out=s_r2, in_=r2[:, :])
            nc.gpsimd.dma_start(out=s_ident, in_=ident[:, :])
            for njg in range(1, NJG):
                cs = slice(njg * 1024, (njg + 1) * 1024)
                nc.gpsimd.dma_start(out=s_ohc[:, cs], in_=ohc[:, cs])
                for kk in range(KT):
                    eng(kk).dma_start(out=s_irT[kk][:, cs], in_=irT[kk, :, cs])

            # --- stat accumulators ---
            st_rmax = stats.tile([128, MI, NJG], F32, name="st_rmax", tag="st0")
            st_rmin = stats.tile([128, MI, NJG], F32, name="st_rmin", tag="st1")
            st_imax = stats.tile([128, 8, MI, NJG], F32, name="st_imax", tag="st2")
            st_imin = stats.tile([128, 8, MI, NJG], F32, name="st_imin", tag="st3")

            def emit_mm_post_chain(njg, mi, P, pend):
                S = emit_post(njg, mi, P)
                pend.append(((njg, mi), S))
                if len(pend) >= 2:
                    (pu, pS) = pend.pop(0)
                    emit_tside(*pu, pS)
                    if pu[1] == MI - 1:
                        emit_stats_out(pu[0])
                return S

            def emit_mm(njg, mi):
                ms = slice(mi * 128, (mi + 1) * 128)
                P = gpool.tile([128, 1024], F32, name="P", tag="P")
                for half in range(2):
                    hs = slice(half * 512, (half + 1) * 512)
                    nj0 = njg * 1024 + half * 512
                    js = slice(nj0, nj0 + 512)
                    for kk in range(KT):
                        nc.tensor.matmul(
                            P[:, hs],
                            lhsT=s_rgbT[:, kk, ms],
                            rhs=s_irT[kk][:, js],
                            start=(kk == 0),
                            stop=False,
                        )
                    nc.tensor.matmul(
                        P[:, hs], lhsT=s_ohr[:, ms], rhs=s_ohc[:, js],
                        start=False, stop=False,
                    )
                    nc.tensor.matmul(
                        P[:, hs], lhsT=s_ones2[:, 0:128], rhs=s_c2hl[:, js],
                        start=False, stop=True,
                    )
                # rgb-side row reduces straight off PSUM (r2 added on host)
                nc.vector.tensor_reduce(
                    out=st_rmax[:, mi, njg : njg + 1], in_=P,
                    axis=mybir.AxisListType.X, op=mybir.AluOpType.max,
                )
                nc.vector.tensor_reduce(
                    out=st_rmin[:, mi, njg : njg + 1], in_=P,
                    axis=mybir.AxisListType.X, op=mybir.AluOpType.min,
                )
                # S = P + ||rgb_i||^2 (ACT bias) -> transpose input
                S = spool.tile([128, 1024], F32, name="S", tag="S", bufs=4)
                nc.scalar.add(S, P, add=s_r2[:, mi : mi + 1])
                return S

            def emit_tside(njg, mi, S):
                T = tpool.tile([128, 8, 128], F32, name="T", tag="T")
                for b in range(8):
                    nc.tensor.transpose(
                        T[:, b, :], S[:, b * 128 : (b + 1) * 128], s_ident
                    )
                # drain T-psum via idle ACT so PE never stalls on DVE pace
                T2 = spool.tile([128, 8, 128], F32, name="T2", tag="T2")
                nc.scalar.copy(T2, T)
                nc.vector.tensor_reduce(
                    out=st_imax[:, :, mi, njg], in_=T2,
                    axis=mybir.AxisListType.X, op=mybir.AluOpType.max,
                )
                nc.vector.tensor_reduce(
                    out=st_imin[:, :, mi, njg], in_=T2,
                    axis=mybir.AxisListType.X, op=mybir.AluOpType.min,
                )

            def emit_stats_out(njg):
                nc.sync.dma_start(
                    out=o_rmax[:, :, njg : njg + 1],
                    in_=st_rmax[:, :, njg : njg + 1],
                )
                nc.sync.dma_start(
                    out=o_rmin[:, :, njg : njg + 1],
                    in_=st_rmin[:, :, njg : njg + 1],
                )
                nc.sync.dma_start(
                    out=o_imax[:, :, :, njg : njg + 1],
                    in_=st_imax[:, :, :, njg : njg + 1],
                )
                nc.sync.dma_start(
                    out=o_imin[:, :, :, njg : njg + 1],
                    in_=st_imin[:, :, :, njg : njg + 1],
                )

            def emit_half(njg, mi, half, P):
                hs = slice(half * 512, (half + 1) * 512)
                nj0 = njg * 1024 + half * 512
                js = slice(nj0, nj0 + 512)
                nc.tensor.matmul(
                    P[:, hs], lhsT=s_ohr[:, mi * 128 : (mi + 1) * 128],
                    rhs=s_ohc[:, js], start=True, stop=False,
                )
                nc.tensor.matmul(
                    P[:, hs], lhsT=s_ones2[:, 0:128], rhs=s_c2hl[:, js],
                    start=False, stop=False,
                )
                for kk in range(KT):
                    nc.tensor.matmul(
                        P[:, hs], lhsT=s_rgbT[:, kk, mi * 128 : (mi + 1) * 128],
                        rhs=s_irT[kk][:, js], start=False, stop=(kk == KT - 1),
                    )

            def emit_post(njg, mi, P):
                nc.vector.tensor_reduce(
                    out=st_rmax[:, mi, njg : njg + 1], in_=P,
                    axis=mybir.AxisListType.X, op=mybir.AluOpType.max,
                )
                nc.vector.tensor_reduce(
                    out=st_rmin[:, mi, njg : njg + 1], in_=P,
                    axis=mybir.AxisListType.X, op=mybir.AluOpType.min,
                )
                S = spool.tile([128, 1024], F32, name="S", tag="S", bufs=4)
                nc.scalar.add(S, P, add=s_r2[:, mi : mi + 1])
                return S

            units = [(njg, mi) for njg in range(NJG) for mi in range(MI)]
            prev = None
            # njg0 in phased pairs: mask/c2 (tiny operands, loaded first) and
            # half0 columns run while the rest of the irT chunks stream in
            pend = []
            for pair in ((0, 1), (2, 3)):
                Ps = {}
                for mi in pair:
                    Ps[mi] = gpool.tile([128, 1024], F32, name="P", tag="P")
                    emit_half(0, mi, 0, Ps[mi])
                for mi in pair:
                    emit_half(0, mi, 1, Ps[mi])
                for mi in pair:
                    S = emit_mm_post_chain(0, mi, Ps[mi], pend)
            for u in units[4:]:
                S = emit_mm(*u)
                pend.append((u, S))
                if len(pend) >= 2:
                    (pu, pS) = pend.pop(0)
                    emit_tside(*pu, pS)
                    if pu[1] == MI - 1:
                        emit_stats_out(pu[0])
            while len(pend) > 1:
                (pu, pS) = pend.pop(0)
                emit_tside(*pu, pS)
                if pu[1] == MI - 1:
                    emit_stats_out(pu[0])
            prev = pend.pop(0)
            # final unit: pipeline the transpose->copy->reduce chain by halves
            fnjg, fmi = prev[0]
            Sf = prev[1]
            for half in range(2):
                Th = tpool.tile([128, 4, 128], F32, name="Th", tag="T", bufs=2)
                for b in range(4):
                    bb = half * 4 + b
                    nc.tensor.transpose(
                        Th[:, b, :], Sf[:, bb * 128 : (bb + 1) * 128], s_ident
                    )
                T2h = spool.tile([128, 4, 128], F32, name="T2h", tag="T2")
                nc.scalar.copy(T2h, Th)
                bs = slice(half * 4, (half + 1) * 4)
                nc.vector.tensor_reduce(
                    out=st_imax[:, bs, fmi, fnjg], in_=T2h,
                    axis=mybir.AxisListType.X, op=mybir.AluOpType.max,
                )
                nc.vector.tensor_reduce(
                    out=st_imin[:, bs, fmi, fnjg], in_=T2h,
                    axis=mybir.AxisListType.X, op=mybir.AluOpType.min,
                )
            emit_stats_out(fnjg)

    nc.compile()
    return nc


def _get_nc():
    if "nc" not in _CACHE:
        _CACHE["nc"] = _build_nc()
    return _CACHE["nc"]


def _round_e8m11(a):
    """Round fp32 array to the float32r (e8m11) grid, RNE."""
    a = np.ascontiguousarray(a, dtype=np.float32)
    u = a.view(np.uint32)
    t = u & np.uint32(0xFFF)
    base = u & np.uint32(0xFFFFF000)
    lsb = (u >> np.uint32(12)) & np.uint32(1)
    up = (t > 0x800) | ((t == 0x800) & (lsb == 1))
    out = base + np.where(up, np.uint32(0x1000), np.uint32(0))
    return out.view(np.float32)


def _maybe_round(a):
    return _round_e8m11(a) if USE_F32R else np.ascontiguousarray(a, np.float32)


def _make_in_maps(inputs, targets):
    x = np.ascontiguousarray(np.asarray(inputs, dtype=np.float32))
    t = np.asarray(targets).astype(np.int64)
    rgb, ir = x[:N], x[N:]
    tr, ti = t[:N], t[N:]

    ir2 = np.einsum("nd,nd->n", ir, ir, dtype=np.float64).astype(np.float32)
    rgb2 = np.einsum("nd,nd->n", rgb, rgb, dtype=np.float64).astype(np.float32)

    lab = np.arange(128)
    irT_np = _maybe_round(np.ascontiguousarray(ir.T)).reshape(KT, 128, N)
    import ml_dtypes
    ohc_np = np.ascontiguousarray(
        (tr[None, :] == lab[:, None]).astype(ml_dtypes.bfloat16)
    )
    c2_hi = ir2.astype(ml_dtypes.bfloat16)
    c2_mid = (ir2 - c2_hi.astype(np.float32)).astype(ml_dtypes.bfloat16)
    c2_lo = (
        ir2 - c2_hi.astype(np.float32) - c2_mid.astype(np.float32)
    ).astype(ml_dtypes.bfloat16)
    c2hl_np = np.stack([c2_hi, c2_mid, c2_lo])  # [3, N] bf16
    ones2_np = np.ones((3, 128), dtype=ml_dtypes.bfloat16)
    ident = np.eye(128, dtype=np.float32)

    in_maps = []
    for k in range(NCORES):
        sl = slice(k * SLAB, (k + 1) * SLAB)
        rgbT_np = _maybe_round(np.ascontiguousarray((-2.0 * rgb[sl]).T)).reshape(
            KT, 128, SLAB
        )
        ohr_np = np.ascontiguousarray(
            ((ti[sl][None, :] == lab[:, None]) * BUMP).astype(ml_dtypes.bfloat16)
        )
        r2_np = np.ascontiguousarray(rgb2[sl].reshape(MI, 128).T)
        in_maps.append(
            {
                "rgbT": rgbT_np,
                "irT": irT_np,
                "ohr": ohr_np,
                "ohc": ohc_np,
                "c2hl": c2hl_np,
                "ones2": ones2_np,
                "r2": r2_np,
                "ident": ident,
            }
        )
    return in_maps, rgb2


def _combine(results, rgb2):
    rgb_mx, rgb_mn = [], []
    for k in range(NCORES):
        rmax = results[k]["rgb_max"].max(axis=2)  # [128, MI] over njg
        rmin = results[k]["rgb_min"].min(axis=2)
        rgb_mx.append(rmax.T.reshape(-1))  # i_local = mi*128+p
        rgb_mn.append(rmin.T.reshape(-1))
    # device rgb stats are missing the per-row ||rgb_i||^2 - add it here
    rgb_mx = np.concatenate(rgb_mx) + rgb2  # [4096]
    rgb_mn = np.concatenate(rgb_mn) + rgb2

    imax = np.max(np.stack([results[k]["ir_max"] for k in range(NCORES)]), axis=0)
    imin = np.min(np.stack([results[k]["ir_min"] for k in range(NCORES)]), axis=0)
    imax = imax.max(axis=2)  # [128, 8, NJG] reduce over mi
    imin = imin.min(axis=2)
    # j = njg*1024 + b*128 + p  ->  [njg, b, p] order
    ir_mx = imax.transpose(2, 1, 0).reshape(-1)  # [4096]
    ir_mn = imin.transpose(2, 1, 0).reshape(-1)

    def side_loss(mx, mn):
        ap = np.sqrt(np.maximum(mx.astype(np.float64) - BUMP, 1e-12))
        an = np.sqrt(np.maximum(mn.astype(np.float64), 1e-12))
        return np.maximum(0.3 - (an - ap), 0.0).mean()

    return np.float32(side_loss(rgb_mx, rgb_mn) + side_loss(ir_mx, ir_mn))


def kernel(inputs, targets):
    global LAST_RESULTS
    nc = _get_nc()
    in_maps, rgb2 = _make_in_maps(inputs, targets)
    res = run_bass_kernel_spmd(nc, in_maps, core_ids=list(range(NCORES)))
    LAST_RESULTS = res
    return _combine(res.results, rgb2)
